# revision 12
# baseline (speedup 1.0000x reference)
"""Trainium2 Bass kernel for the differentiable circle renderer.

Math: the sequential over-composite
    canvas <- canvas*(1-g_i) + col_i*g_i,   g_i = alpha_i * sigmoid((r_i-d_i)/0.01)
unrolls (Abel summation) to
    canvas_c = K_c + sum_i D_ic * S_i,      S_i = prod_{j>=i} (1-g_j)
with D_0c = 1-col_0c, D_ic = col_{i-1,c}-col_ic (i>=1), K_c = col_{N-1,c}.
Since g_j = alpha_j*m_j < 1 strictly (alpha ~ U[0,1)), suffix products go
through log space: S_i = exp(sum_{j>=i} ln(1-g_j)), and suffix sums are a
triangular matmul on the TensorEngine.

Layout: circles (N=128) on SBUF partitions, pixels on the free dim.  Each of
8 cores owns 128 canvas rows.  Per row y:
    dist = Sqrt(U + V[:,y])            (ACT, per-partition bias)
    m    = Sigmoid(-100*dist + 100*r)  (ACT)
    L    = Ln(-alpha*m + 1)            (ACT, per-partition scale)
    SL   = Tri @ L                     (PE, fp16 hi/lo split -> fp32 PSUM)
    S    = Exp(SL)                     (ACT)
    out  = D @ S                       (PE, fp16 D hi/lo)  [+ K_c on host]
U[i,x] = (x-cx_i)^2 and V[i,y] = (y-cy_i)^2 are host-precomputed.
ACT table sets force phase-blocking: sqrt / sigmoid / {ln,exp} grouped over
blocks of R rows to amortize the 1.28us table reloads.
"""

import sys

sys.path.insert(0, "/opt/trn_rl_repo")

import numpy as np

CANVAS = 1024
N = 128
NCORES = 8
ROWS = CANVAS // NCORES  # 128 rows per core
W = CANVAS

_CACHE = {}


def split_multiwaits(nc, max_waits=1):
    """Walrus in this container rejects >max_waits sem waits on one
    instruction; hoist extras onto standalone NoOps placed just before."""
    from concourse import mybir

    ctr = 0
    for bb in nc.main_func.blocks:
        new = []
        for inst in bb.instructions:
            si = inst.sync_info
            if si is not None and len(si.on_wait) > max_waits:
                waits = list(si.on_wait)
                extra, keep = waits[:-max_waits], waits[-max_waits:]
                for wt in extra:
                    ctr += 1
                    nop = mybir.InstNoOp(
                        name=f"waitsplit_{ctr}",
                        opcode="NoOp",
                        engine=inst.engine,
                        sync_info=mybir.SyncInfo(on_wait=[wt], on_update=[]),
                    )
                    new.append(nop)
                inst.sync_info = mybir.SyncInfo(
                    on_wait=keep, on_update=list(si.on_update)
                )
            new.append(inst)
        bb.instructions = new
    return ctr


def build_nc(R=16, split=True, l_lo=False):
    """Build the SPMD Bass program (identical on all cores; data differs)."""
    import concourse.bass as bass
    import concourse.tile as tile
    from concourse import mybir

    f32 = mybir.dt.float32
    f16 = mybir.dt.float16
    AF = mybir.ActivationFunctionType

    nc = bass.Bass()
    U_d = nc.declare_dram_parameter("U", [N, W], f32, isOutput=False)
    V_d = nc.declare_dram_parameter("V", [N, ROWS], f32, isOutput=False)
    BS_d = nc.declare_dram_parameter("BS", [N, 1], f32, isOutput=False)
    NA_d = nc.declare_dram_parameter("NA", [N, 1], f32, isOutput=False)
    TRI_d = nc.declare_dram_parameter("TRI", [N, N], f16, isOutput=False)
    DH_d = nc.declare_dram_parameter("DH", [N, 3], f16, isOutput=False)
    DL_d = nc.declare_dram_parameter("DL", [N, 3], f16, isOutput=False)
    KC_d = nc.declare_dram_parameter("KC", [3, 1], f32, isOutput=False)
    OUT_d = nc.declare_dram_parameter("OUT", [3, ROWS, W], f32, isOutput=True)

    with tile.TileContext(nc) as tc:
        with (
            tc.tile_pool(name="const", bufs=1) as cpool,
            tc.tile_pool(name="work", bufs=R // 4 + 2) as wpool,
            tc.tile_pool(name="l16", bufs=3) as lpool,
            tc.tile_pool(name="spool", bufs=3) as spool,
            tc.tile_pool(name="stage", bufs=3) as stpool,
            tc.tile_pool(name="sl", bufs=1, space="PSUM") as slpool,
            tc.tile_pool(name="ob", bufs=1, space="PSUM") as opool,
        ):
            U = cpool.tile([N, W], f32)
            nc.gpsimd.dma_start(U[:], U_d[:])
            V = cpool.tile([N, ROWS], f32)
            nc.gpsimd.dma_start(V[:], V_d[:])
            BS = cpool.tile([N, 1], f32)
            nc.gpsimd.dma_start(BS[:], BS_d[:])
            NA = cpool.tile([N, 1], f32)
            nc.gpsimd.dma_start(NA[:], NA_d[:])
            TRI = cpool.tile([N, N], f16)
            nc.gpsimd.dma_start(TRI[:], TRI_d[:])
            DH = cpool.tile([N, 3], f16)
            nc.gpsimd.dma_start(DH[:], DH_d[:])
            DL = cpool.tile([N, 3], f16)
            nc.gpsimd.dma_start(DL[:], DL_d[:])
            KC = cpool.tile([3, 1], f32)
            nc.gpsimd.dma_start(KC[:], KC_d[:])

            Q = 4  # rows per ACT op (quad)
            for blk in range(ROWS // R):
                r0 = blk * R
                quads = []
                # Phase A0 (DVE): d2 = U + V[:,r] per row into quad tiles,
                # then Phase A (ACT, table sqrt): dist = sqrt(d2), one op/quad
                for p in range(R // Q):
                    t = wpool.tile([N, Q * W], f32, tag="chain")
                    quads.append(t)
                    for j in range(Q):
                        r = r0 + Q * p + j
                        nc.vector.tensor_scalar_add(
                            t[:, j * W : (j + 1) * W], U[:], V[:, r : r + 1]
                        )
                    nc.scalar.activation(t[:], t[:], AF.Sqrt, bias=0.0, scale=1.0)
                # Phase B (table sigmoid): m = sigmoid(-100*dist + 100*r)
                for p in range(R // Q):
                    t = quads[p]
                    nc.scalar.activation(
                        t[:], t[:], AF.Sigmoid, bias=BS[:, 0:1], scale=-100.0
                    )
                # Phase C (table ln+exp): L = ln(1 - alpha*m) -> fp16;
                # SL = Tri@L (PE); S = exp(SL) -> fp16; out = D@S (PE)
                for p in range(R // Q):
                    t = quads[p]
                    l16 = lpool.tile([N, Q * W], f16, tag="l16")
                    nc.scalar.activation(
                        l16[:], t[:], AF.Ln, scale=NA[:, 0:1], bias=1.0
                    )
                    for half in (0, 1):  # pair of rows within the quad
                        ob = opool.tile([3, 4 * 512], f32)  # 4 PSUM banks
                        sl = slpool.tile([N, 2 * W], f32)  # 4 PSUM banks
                        for j in (0, 1):
                            for h in (0, 1):
                                seg = slice(
                                    (2 * half + j) * W + h * 512,
                                    (2 * half + j) * W + (h + 1) * 512,
                                )
                                dst = sl[:, j * W + h * 512 : j * W + (h + 1) * 512]
                                nc.tensor.matmul(
                                    dst, TRI[:], l16[:, seg], start=True, stop=True
                                )
                        s16 = spool.tile([N, 2 * W], f16, tag="s16")
                        nc.scalar.activation(s16[:], sl[:], AF.Exp)
                        for j in (0, 1):
                            for h in (0, 1):
                                q = 2 * j + h
                                nc.tensor.matmul(
                                    ob[:, q * 512 : (q + 1) * 512],
                                    DH[:],
                                    s16[:, j * W + h * 512 : j * W + (h + 1) * 512],
                                    start=True,
                                    stop=False,
                                )
                        for j in (0, 1):
                            for h in (0, 1):
                                q = 2 * j + h
                                nc.tensor.matmul(
                                    ob[:, q * 512 : (q + 1) * 512],
                                    DL[:],
                                    s16[:, j * W + h * 512 : j * W + (h + 1) * 512],
                                    start=False,
                                    stop=True,
                                )
                        stage = stpool.tile([3, 4 * 512], f32)
                        nc.vector.tensor_scalar_add(stage[:], ob[:], KC[:, 0:1])
                        r = r0 + Q * p + 2 * half
                        nc.sync.dma_start(OUT_d[:, r : r + 2, :], stage[:])
    if split:
        split_multiwaits(nc)
    return nc


def host_inputs(centers, radii, colors):
    """Per-core input maps + the host-side additive constant K_c."""
    centers = np.asarray(centers, np.float32)
    radii = np.asarray(radii, np.float32)
    colors = np.asarray(colors, np.float32)
    xs = np.linspace(0.0, 1.0, W, dtype=np.float32)
    ys = np.linspace(0.0, 1.0, CANVAS, dtype=np.float32)
    cx = centers[:, 0]
    cy = centers[:, 1]
    U = (xs[None, :] - cx[:, None]) ** 2  # [N, W] f32
    BS = (100.0 * radii)[:, None].astype(np.float32)
    NA = (-colors[:, 3])[:, None].astype(np.float32)
    rgb = colors[:, :3].astype(np.float64)
    D = np.empty((N, 3), np.float64)
    D[0] = 1.0 - rgb[0]
    D[1:] = rgb[:-1] - rgb[1:]
    DH = D.astype(np.float16)
    DL = (D - DH.astype(np.float64)).astype(np.float16)
    TRI = np.tril(np.ones((N, N), np.float16))  # TRI[j,i]=1 iff j>=i
    Kc = rgb[-1].astype(np.float32)

    in_maps = []
    for k in range(NCORES):
        ys_k = ys[k * ROWS : (k + 1) * ROWS]
        Vk = (ys_k[None, :] - cy[:, None]) ** 2  # [N, ROWS]
        in_maps.append(
            {
                "U": np.ascontiguousarray(U, np.float32),
                "V": np.ascontiguousarray(Vk, np.float32),
                "BS": BS,
                "NA": NA,
                "TRI": TRI,
                "DH": DH,
                "DL": DL,
                "KC": Kc.reshape(3, 1).astype(np.float32),
            }
        )
    return in_maps, Kc


def kernel(centers, radii, colors, trace=False):
    from concourse.bass_utils import run_bass_kernel_spmd

    if "nc" not in _CACHE:
        _CACHE["nc"] = build_nc()
    nc = _CACHE["nc"]
    in_maps, Kc = host_inputs(centers, radii, colors)
    res = run_bass_kernel_spmd(nc, in_maps, list(range(NCORES)), trace=trace)
    _CACHE["last_result"] = res
    parts = [res.results[k]["OUT"] for k in range(NCORES)]
    out = np.concatenate(parts, axis=1)
    return np.ascontiguousarray(out, dtype=np.float32)
